# revision 49
# baseline (speedup 1.0000x reference)
"""Trainium2 Bass kernel for EnhancedMetaWeightNetwork.

Full (unsharded) inputs in, full output out. Internally: 8 NeuronCores,
core c handles batch b = c // 2 and query-row half c % 2 (1024 rows).
Attention K/V are computed per-core for the full sequence of the core's
batch (duplicated across the 2 cores sharing a batch; no collectives).

v3 layout strategy (bf16 storage, fp32 PSUM accumulation):
  - x = hidden + pos_embed precomputed on host; uploaded as bf16 x^T.
  - out_w folded into w1 on host (w1eff = w1[:, H:] @ out_w), so the
    attention out-projection disappears; h1 = w1x @ x + w1eff @ ctx.
  - everything SBUF-resident: no DRAM scratch.
  - fused per-head pipeline: K[0],Q[0] -> V (all) -> for each head:
    attention(h) interleaved with K/Q(h+1), so the scalar-engine exp
    stream hides under the PE matmul stream.
  - scoresT [key, query] per head in a 2-bank PSUM tile covering the
    full 1024 own queries; one exp activation per key-tile.
  - softmax denominator: pairwise ex adds on gpsimd + running
    accumulation on DVE, one ones-matmul broadcast, then
    reciprocal_approx_fast.
  - K/Q staging (PSUM -> bf16 SBUF + bias) on DVE to keep scalar free
    for exp; all constants packed into one DMA.
  - importance lookup via indirect DMA gather from the vocab table.
"""

import numpy as np

H = 1024
NH = 8
HD = 128           # head dim
S = 2048           # keys / full sequence
SQ = 1024          # own query rows per core
MD = 256           # meta dim
MD2 = 128
VOCAB = 32000
MIN_W, MAX_W = 0.1, 5.0
LN_EPS = 1e-5
P = 128
INV_SQRT_HD = 1.0 / np.sqrt(np.float32(HD))

# packed fp32 constant columns: [kbias 16 | maskf 8 | bq 8 | bk 8 | b1 2 |
#  g1 2 | be1 2 | b3 1 | w3 128 | b2 128 | g2 128 | be2 128 | bv 1024]
_CPK_SPANS = {}
_off = 0
for _name, _n in [("kbias", 16), ("maskf", 8), ("bq", 8), ("bk", 8),
                  ("b1", 2), ("g1", 2), ("be1", 2), ("b3", 1), ("w3", MD2),
                  ("b2", MD2), ("g2", MD2), ("be2", MD2), ("bv", H)]:
    _CPK_SPANS[_name] = (_off, _off + _n)
    _off += _n
NCPK = _off

_CACHE = {}


def _build():
    import concourse.bass as bass
    import concourse.mybir as mybir
    import concourse.tile as tile
    from concourse import bacc

    f32 = mybir.dt.float32
    bf16 = mybir.dt.bfloat16
    i32 = mybir.dt.int32
    OP = mybir.AluOpType
    ACT = mybir.ActivationFunctionType

    nc = bacc.Bacc("TRN2", target_bir_lowering=False, debug=False,
                   enable_asserts=False, num_devices=8)

    # ---------------- DRAM parameters ----------------
    dp = nc.declare_dram_parameter
    # all weight/activation uploads pre-permuted on host so every DMA source
    # is contiguous (partition-major [p, chunk, cols])
    xTr = dp("xTr", [S // 512, P, H // P, 512], bf16, isOutput=False)
    wqR = dp("wqR", [NH, P, H // P, P], bf16, isOutput=False)
    wkR = dp("wkR", [NH, P, H // P, P], bf16, isOutput=False)
    wvR = dp("wvR", [P, H // P, H], bf16, isOutput=False)
    w1cR = dp("w1cR", [P, 2 * H // P, MD], bf16, isOutput=False)
    w2R = dp("w2R", [P, MD // P, MD2], bf16, isOutput=False)
    cpack = dp("cpack", [P, NCPK], f32, isOutput=False)
    tokc = dp("tokc", [P, SQ // P], i32, isOutput=False)
    table = dp("table", [VOCAB, 1], f32, isOutput=False)
    out = dp("out", [SQ], f32, isOutput=True)

    NKT = S // P          # 16 key tiles
    NC8 = H // P          # 8 feature chunks
    NTT = SQ // P         # 8 own token tiles
    NFT = MD // P         # 2 feature tiles of h1

    with tile.TileContext(nc) as tc:
        with tc.tile_pool(name="const", bufs=1) as cst, \
             tc.tile_pool(name="persist", bufs=1) as pa:

            # ---------------- constants (no DMA) ----------------
            ones_f = cst.tile([P, P], f32, tag="ones_f")
            nc.any.memset(ones_f[:], 1.0)
            ones_b = cst.tile([P, P], bf16, tag="ones_b")
            nc.vector.tensor_copy(ones_b[:], ones_f[:])
            eps_sb = cst.tile([P, 1], f32, tag="eps")
            nc.any.memset(eps_sb[:], LN_EPS)

            # ---- long-lived activations ----
            x_sb = pa.tile([P, NC8, S], bf16, tag="x")       # full x^T (own cols first)
            ctx_sb = pa.tile([P, NC8, SQ], bf16, tag="ctx")  # ctx^T (head-major)
            v_sb = pa.tile([P, NKT, H], bf16, tag="v")
            w1c_sb = pa.tile([P, 2 * NC8, MD], bf16, tag="w1c")
            h1p = pa.tile([P, NFT, SQ], bf16, tag="h1p")
            h1n = pa.tile([P, NFT, SQ], bf16, tag="h1n")
            res_sb = pa.tile([P, NTT], f32, tag="res")

            with tc.tile_pool(name="kqs", bufs=3) as kqs, \
                 tc.tile_pool(name="wst", bufs=2) as wst, \
                 tc.tile_pool(name="ps_kq", bufs=2, space="PSUM") as ps_kq:

                # ---- DMA issue order: head-0 weights, then x, then rest ----
                def load_w(src, h, tag):
                    t = wst.tile([P, NC8, P], bf16, tag=tag)
                    nc.sync.dma_start(t[:], src[h])
                    return t

                wk0 = load_w(wkR, 0, "wk")
                wq0 = load_w(wqR, 0, "wq")

                # load x by column blocks so K(0) can start after the first
                def load_x(cb):
                    nc.sync.dma_start(
                        x_sb[:, :, cb * 512:(cb + 1) * 512], xTr[cb])

                # small constants (biases etc.) before x so K staging never
                # waits; the big bv tail of cpack comes later
                cpk = cst.tile([P, NCPK], f32, tag="cpk")
                _bv_lo = _CPK_SPANS["bv"][0]
                nc.sync.dma_start(cpk[:, 0:_bv_lo], cpack[:, 0:_bv_lo])

                load_x(0)
                load_x(1)

                def cslice(name):
                    lo, hi = _CPK_SPANS[name]
                    return cpk[:, lo:hi]

                kbias_sb = cslice("kbias")
                maskf_sb = cslice("maskf")
                bq_sb = cslice("bq")
                bk_sb = cslice("bk")
                b1_c = cslice("b1")
                g1_c = cslice("g1")
                be1_c = cslice("be1")
                b3_sb = cslice("b3")
                w3_sb = cslice("w3")
                b2_sb = cslice("b2")
                g2_sb = cslice("g2")
                be2_sb = cslice("be2")
                bv_sb = cslice("bv")

                # ---- per-head K/Q projection + staging (DVE) ----
                def emit_k(h, wk_sb, k_h=None, sbs=None):
                    if k_h is None:
                        k_h = kqs.tile([P, S], bf16, tag="k_h")
                    for sb in (sbs if sbs is not None else range(S // 512)):
                        psk = ps_kq.tile([P, 512], mybir.dt.float32, tag="kq",
                                         name="psk")
                        for c8 in range(NC8):
                            nc.tensor.matmul(psk[:], lhsT=wk_sb[:, c8, :],
                                             rhs=x_sb[:, c8, sb * 512:(sb + 1) * 512],
                                             start=(c8 == 0), stop=(c8 == NC8 - 1))
                        nc.vector.tensor_tensor(
                            out=k_h[:, sb * 512:(sb + 1) * 512], in0=psk[:],
                            in1=bk_sb[:, h:h + 1].to_broadcast([P, 512]),
                            op=OP.add)
                    return k_h

                def emit_q(h, wq_sb):
                    q_h = kqs.tile([P, SQ], bf16, tag="q_h")
                    for qb in range(SQ // 512):
                        psq = ps_kq.tile([P, 512], mybir.dt.float32, tag="kq",
                                         name="psq")
                        for c8 in range(NC8):
                            nc.tensor.matmul(psq[:], lhsT=wq_sb[:, c8, :],
                                             rhs=x_sb[:, c8, qb * 512:(qb + 1) * 512],
                                             start=(c8 == 0), stop=(c8 == NC8 - 1))
                        nc.vector.tensor_tensor(
                            out=q_h[:, qb * 512:(qb + 1) * 512], in0=psq[:],
                            in1=bq_sb[:, h:h + 1].to_broadcast([P, 512]),
                            op=OP.add)
                    return q_h

                # ---- V weight / remaining loads interleaved with the
                # two up-front heads of K/Q (fills the x load window) ----
                with tc.tile_pool(name="wvp", bufs=1) as wvp, \
                     tc.tile_pool(name="ps_v", bufs=3, space="PSUM") as ps_v:
                    wv_sb = wvp.tile([P, NC8, H], bf16, tag="wv")
                    nc.sync.dma_start(wv_sb[:], wvR[:])
                    load_x(2)
                    load_x(3)
                    wk1 = load_w(wkR, 1, "wk")
                    wq1 = load_w(wqR, 1, "wq")
                    nc.sync.dma_start(cpk[:, _bv_lo:], cpack[:, _bv_lo:])
                    tok_sb = cst.tile([P, NTT], i32, tag="tok")
                    nc.sync.dma_start(tok_sb[:], tokc[:])
                    w2_sb = cst.tile([P, NFT, MD2], bf16, tag="w2")
                    nc.sync.dma_start(w2_sb[:], w2R[:])
                    nc.sync.dma_start(w1c_sb[:], w1cR[:])

                    # importance gather (gpsimd queue; independent)
                    imp_all = cst.tile([P, NTT], f32, tag="imp_all")
                    for tt in range(NTT):
                        nc.gpsimd.indirect_dma_start(
                            out=imp_all[:, tt:tt + 1], out_offset=None,
                            in_=table[:],
                            in_offset=bass.IndirectOffsetOnAxis(
                                ap=tok_sb[:, tt:tt + 1], axis=0))

                    # head 0/1 K/Q; K0's other-half blocks after Q0 so the
                    # PE is not waiting on x blocks 2/3
                    k_next = emit_k(0, wk0, sbs=[0, 1])
                    q_next = emit_q(0, wq0)
                    emit_k(0, wk0, k_h=k_next, sbs=[2, 3])
                    k_next2 = emit_k(1, wk1)
                    q_next2 = emit_q(1, wq1)

                    for tt in range(NKT):
                        psv = ps_v.tile([P, 1024], mybir.dt.float32, tag="v2")
                        for c8 in range(NC8):
                            lhsT = x_sb[:, c8, tt * P:(tt + 1) * P]
                            for db in range(H // 512):
                                nc.tensor.matmul(psv[:, db * 512:(db + 1) * 512],
                                                 lhsT=lhsT,
                                                 rhs=wv_sb[:, c8, db * 512:(db + 1) * 512],
                                                 start=(c8 == 0), stop=(c8 == NC8 - 1))
                        nc.vector.tensor_tensor(out=v_sb[:, tt, :], in0=psv[:],
                                                in1=bv_sb[:], op=OP.add)

                # ---- fused attention + next-head K/Q pipeline ----
                with tc.tile_pool(name="exps", bufs=4) as exps, \
                     tc.tile_pool(name="trp", bufs=3) as trp, \
                     tc.tile_pool(name="trd", bufs=2) as trd, \
                     tc.tile_pool(name="rcps", bufs=2) as rcps, \
                     tc.tile_pool(name="ps_sc", bufs=2, space="PSUM") as ps_sc, \
                     tc.tile_pool(name="ps_cps", bufs=1, space="PSUM") as ps_cps:
                    for h in range(NH):
                        k_cur, q_cur = k_next, q_next
                        k_next, q_next = k_next2, q_next2
                        if h + 2 < NH:
                            wk_nx = load_w(wkR, h + 2, "wk")
                            wq_nx = load_w(wqR, h + 2, "wq")

                        cps = ps_cps.tile([P, 1024], mybir.dt.float32, tag="cps")
                        pair = None
                        acc = None
                        for kt in range(NKT):
                            sc = ps_sc.tile([P, 1024], mybir.dt.float32, tag="sc")
                            for qb in range(SQ // 512):
                                nc.tensor.matmul(
                                    sc[:, qb * 512:(qb + 1) * 512],
                                    lhsT=k_cur[:, kt * P:(kt + 1) * P],
                                    rhs=q_cur[:, qb * 512:(qb + 1) * 512],
                                    start=True, stop=True)
                            ex = exps.tile([P, 1024], bf16, tag="ex")
                            nc.scalar.activation(ex[:], sc[:], ACT.Exp,
                                                 bias=kbias_sb[:, kt:kt + 1],
                                                 scale=1.0)
                            for qb in range(SQ // 512):
                                nc.tensor.matmul(
                                    cps[:, qb * 512:(qb + 1) * 512],
                                    lhsT=v_sb[:, kt, h * P:(h + 1) * P],
                                    rhs=ex[:, qb * 512:(qb + 1) * 512],
                                    start=(kt == 0), stop=(kt == NKT - 1))
                            if kt % 2 == 0:
                                ex_even = ex
                            else:
                                # pairwise add + running accumulation on DVE
                                pair = trp.tile([P, 1024], bf16, tag="pair")
                                nc.vector.tensor_tensor(out=pair[:], in0=ex_even[:],
                                                        in1=ex[:], op=OP.add)
                                if kt == 1:
                                    acc = pair
                                else:
                                    nacc = trd.tile([P, 1024], bf16, tag="acc")
                                    nc.vector.tensor_tensor(out=nacc[:], in0=acc[:],
                                                            in1=pair[:], op=OP.add)
                                    acc = nacc

                        # denominator broadcast via ones-matmul, then approx
                        # reciprocal and context scale (emitted after K(h+1)
                        # so the PE never waits on the adder tree)
                        def finish_head(h, cps, acc):
                            rcb = rcps.tile([P, 1024], f32, tag="rcb")
                            for qb in range(SQ // 512):
                                dnf = ps_kq.tile([P, 512], mybir.dt.float32,
                                                 tag="kq", name="dnf")
                                nc.tensor.matmul(dnf[:], lhsT=ones_b[:],
                                                 rhs=acc[:, qb * 512:(qb + 1) * 512],
                                                 start=True, stop=True)
                                with nc.allow_low_precision(reason="softmax rcp"):
                                    nc.vector.reciprocal_approx_fast(
                                        out=rcb[:, qb * 512:(qb + 1) * 512],
                                        in_=dnf[:])
                            nc.vector.tensor_tensor(out=ctx_sb[:, h, :], in0=cps[:],
                                                    in1=rcb[:], op=OP.mult)

                        if h + 2 < NH:
                            k_next2 = emit_k(h + 2, wk_nx)
                            finish_head(h, cps, acc)
                            q_next2 = emit_q(h + 2, wq_nx)
                        else:
                            finish_head(h, cps, acc)

            # ---------- meta MLP ----------
            with tc.tile_pool(name="mw", bufs=1) as mw, \
                 tc.tile_pool(name="msml", bufs=2) as sml, \
                 tc.tile_pool(name="ps_m", bufs=6, space="PSUM") as ps2:
                # ---- h1 in feature-major: h1preT [256, SQ] (qb-outer so
                # LN1(qb0) overlaps the qb1 matmuls on the PE) ----
                for qb in range(SQ // 512):
                    qsl = slice(qb * 512, (qb + 1) * 512)
                    for ft in range(NFT):
                        psf_t = ps2.tile([P, 512], mybir.dt.float32,
                                         tag="mm512", name="psf")
                        for j in range(2 * NC8):
                            if j < NC8:
                                rhs = x_sb[:, j, qsl]
                            else:
                                rhs = ctx_sb[:, j - NC8, qsl]
                            nc.tensor.matmul(
                                psf_t[:],
                                lhsT=w1c_sb[:, j, ft * P:(ft + 1) * P],
                                rhs=rhs,
                                start=(j == 0), stop=(j == 2 * NC8 - 1))
                        nc.scalar.activation(
                            h1p[:, ft, qsl], psf_t[:],
                            ACT.Identity, bias=b1_c[:, ft:ft + 1], scale=1.0)
                    h1sqs = []
                    for ft in range(NFT):
                        h1sq = sml.tile([P, 512], bf16, tag=f"h1sq{ft}")
                        nc.vector.tensor_tensor(out=h1sq[:], in0=h1p[:, ft, qsl],
                                                in1=h1p[:, ft, qsl], op=OP.mult)
                        h1sqs.append(h1sq)
                    psA = ps2.tile([P, 512], mybir.dt.float32, tag="mm512",
                                   name="psA")
                    psB = ps2.tile([P, 512], mybir.dt.float32, tag="mm512",
                                   name="psB")
                    for ft in range(NFT):
                        nc.tensor.matmul(psA[:], lhsT=ones_b[:],
                                         rhs=h1p[:, ft, qsl],
                                         start=(ft == 0), stop=(ft == NFT - 1))
                    for ft in range(NFT):
                        nc.tensor.matmul(psB[:], lhsT=ones_b[:],
                                         rhs=h1sqs[ft][:],
                                         start=(ft == 0), stop=(ft == NFT - 1))
                    nmean = sml.tile([P, 512], f32, tag="nmean")
                    ex2m = sml.tile([P, 512], f32, tag="ex2m")
                    m2r = sml.tile([P, 512], f32, tag="m2r")
                    nc.vector.tensor_scalar_mul(nmean[:], psA[:], -1.0 / MD)
                    nc.vector.tensor_scalar_mul(ex2m[:], psB[:], 1.0 / MD)
                    nc.vector.tensor_tensor(out=m2r[:], in0=nmean[:],
                                            in1=nmean[:], op=OP.mult)
                    nc.vector.tensor_tensor(out=ex2m[:], in0=ex2m[:],
                                            in1=m2r[:], op=OP.subtract)
                    # rstd = exp(-0.5 * ln(var + eps)) on ACT (fast path)
                    nc.scalar.activation(ex2m[:], ex2m[:], ACT.Ln,
                                         bias=eps_sb[:, 0:1], scale=1.0)
                    rstd = sml.tile([P, 512], bf16, tag="rstd")
                    with nc.allow_low_precision(reason="bf16 layernorm scale"):
                        nc.scalar.activation(rstd[:], ex2m[:], ACT.Exp,
                                             bias=0.0, scale=-0.5)
                    for ft in range(NFT):
                        h1c = sml.tile([P, 512], bf16, tag=f"h1c{ft}")
                        nc.vector.tensor_tensor(out=h1c[:], in0=h1p[:, ft, qsl],
                                                in1=nmean[:], op=OP.add)
                        nc.vector.tensor_tensor(out=h1c[:], in0=h1c[:],
                                                in1=rstd[:], op=OP.mult)
                        nc.scalar.activation(h1n[:, ft, qsl], h1c[:],
                                             ACT.Relu, bias=be1_c[:, ft:ft + 1],
                                             scale=g1_c[:, ft:ft + 1])

                # ---- h2 + LN2/final, per-token-tile groups so the LN2
                # chains pipeline behind the h2 matmuls (short tail) ----
                F2 = float(MD2)
                NG = 2
                GT = NTT // NG      # 4 token tiles per group
                hb2s = []
                for g in range(NG):
                    hb2 = mw.tile([P, GT, MD2], f32, tag=f"hb2{g}")
                    for ti in range(GT):
                        tt = g * GT + ti
                        ph2_t = ps2.tile([P, 512], mybir.dt.float32, tag="mm512",
                                         name="ph2")
                        ph2 = ph2_t[:, :MD2]
                        for ft in range(NFT):
                            nc.tensor.matmul(ph2,
                                             lhsT=h1n[:, ft, tt * P:(tt + 1) * P],
                                             rhs=w2_sb[:, ft, :],
                                             start=(ft == 0), stop=(ft == NFT - 1))
                        nc.vector.scalar_tensor_tensor(out=hb2[:, ti, :], in0=ph2,
                                                       scalar=1.0, in1=b2_sb[:],
                                                       op0=OP.mult, op1=OP.add)
                    hb2s.append(hb2)
                # LN2/final chains for the two groups, op-interleaved so the
                # DVE pipeline hides each chain's dependency latency
                R = range(NG)
                def gtile(name, shape):
                    return [sml.tile(shape, f32, tag=f"{name}{g}",
                                     name=f"{name}{g}") for g in R]
                sums2 = gtile("sums2", [P, GT])
                msq = gtile("msq", [P, GT, MD2])
                ssq2 = gtile("ssq2", [P, GT])
                nm2 = gtile("nm2", [P, GT])
                ex22 = gtile("ex22", [P, GT])
                mm2v = gtile("mm2", [P, GT])
                var2 = gtile("var2", [P, GT])
                std2 = gtile("std2", [P, GT])
                rstd2 = gtile("rstd2", [P, GT])
                t1a = gtile("t1a", [P, GT, MD2])
                base8 = gtile("base8", [P, GT])
                imp1a = gtile("imp1a", [P, GT])
                for g in R:
                    nc.vector.tensor_scalar_add(imp1a[g][:],
                                                imp_all[:, g * GT:(g + 1) * GT],
                                                1.0)
                for g in R:
                    nc.vector.reduce_sum(sums2[g][:], hb2s[g][:],
                                         axis=mybir.AxisListType.X)
                for g in R:
                    nc.vector.tensor_tensor(out=msq[g][:], in0=hb2s[g][:],
                                            in1=hb2s[g][:], op=OP.mult)
                for g in R:
                    nc.vector.reduce_sum(ssq2[g][:], msq[g][:],
                                         axis=mybir.AxisListType.X)
                for g in R:
                    nc.vector.tensor_scalar_mul(nm2[g][:], sums2[g][:], -1.0 / F2)
                for g in R:
                    nc.vector.tensor_scalar_mul(ex22[g][:], ssq2[g][:], 1.0 / F2)
                for g in R:
                    nc.vector.tensor_tensor(out=mm2v[g][:], in0=nm2[g][:],
                                            in1=nm2[g][:], op=OP.mult)
                for g in R:
                    nc.vector.tensor_tensor(out=var2[g][:], in0=ex22[g][:],
                                            in1=mm2v[g][:], op=OP.subtract)
                for g in R:
                    nc.scalar.activation(std2[g][:], var2[g][:], ACT.Sqrt,
                                         bias=eps_sb[:, 0:1], scale=1.0)
                for g in R:
                    nc.vector.reciprocal(rstd2[g][:], std2[g][:])
                for g in R:
                    nc.vector.tensor_tensor(
                        out=t1a[g][:], in0=hb2s[g][:],
                        in1=nm2[g][:, :, None].to_broadcast([P, GT, MD2]),
                        op=OP.add)
                for g in R:
                    nc.vector.tensor_tensor(
                        out=t1a[g][:], in0=t1a[g][:],
                        in1=rstd2[g][:, :, None].to_broadcast([P, GT, MD2]),
                        op=OP.mult)
                for g in R:
                    nc.vector.tensor_tensor(
                        out=t1a[g][:], in0=t1a[g][:],
                        in1=g2_sb[:, None, :].to_broadcast([P, GT, MD2]),
                        op=OP.mult)
                for g in R:
                    nc.vector.tensor_tensor(
                        out=t1a[g][:], in0=t1a[g][:],
                        in1=be2_sb[:, None, :].to_broadcast([P, GT, MD2]),
                        op=OP.add)
                for g in R:
                    nc.vector.tensor_scalar_max(t1a[g][:], t1a[g][:], 0.0)
                for g in R:
                    nc.vector.tensor_tensor(
                        out=t1a[g][:], in0=t1a[g][:],
                        in1=w3_sb[:, None, :].to_broadcast([P, GT, MD2]),
                        op=OP.mult)
                for g in R:
                    nc.vector.reduce_sum(base8[g][:], t1a[g][:],
                                         axis=mybir.AxisListType.X)
                for g in R:
                    nc.vector.tensor_tensor(
                        out=base8[g][:], in0=base8[g][:],
                        in1=b3_sb[:, 0:1].to_broadcast([P, GT]), op=OP.add)
                for g in R:
                    nc.vector.tensor_tensor(out=base8[g][:], in0=base8[g][:],
                                            in1=imp1a[g][:], op=OP.mult)
                for g in R:
                    nc.vector.tensor_scalar(base8[g][:], base8[g][:], MAX_W,
                                            MIN_W, op0=OP.min, op1=OP.max)
                for g in R:
                    nc.vector.tensor_tensor(
                        out=res_sb[:, g * GT:(g + 1) * GT], in0=base8[g][:],
                        in1=maskf_sb[:, g * GT:(g + 1) * GT], op=OP.mult)
                nc.sync.dma_start(out[:].rearrange("(t p) -> p t", p=P),
                                  res_sb[:])

    nc.compile()
    return nc


def _get_program():
    if "nc" not in _CACHE:
        _CACHE["nc"] = _build()
    return _CACHE["nc"]


def _prep_in_maps(inputs):
    import ml_dtypes
    bf = ml_dtypes.bfloat16

    hidden = np.asarray(inputs["hidden_states"], dtype=np.float32)
    token_ids = np.asarray(inputs["token_ids"], dtype=np.int32)
    mask = np.asarray(inputs["attention_mask"]).astype(bool)
    pos = np.asarray(inputs["pos_embed"], dtype=np.float32)
    in_proj_w = np.asarray(inputs["in_proj_w"], dtype=np.float32)
    in_proj_b = np.asarray(inputs["in_proj_b"], dtype=np.float32)
    out_w = np.asarray(inputs["out_w"], dtype=np.float32)
    out_b = np.asarray(inputs["out_b"], dtype=np.float32)
    w1 = np.asarray(inputs["w1"], dtype=np.float32)
    b1 = np.asarray(inputs["b1"], dtype=np.float32)
    g1 = np.asarray(inputs["g1"], dtype=np.float32)
    beta1 = np.asarray(inputs["beta1"], dtype=np.float32)
    w2 = np.asarray(inputs["w2"], dtype=np.float32)
    b2 = np.asarray(inputs["b2"], dtype=np.float32)
    g2 = np.asarray(inputs["g2"], dtype=np.float32)
    beta2 = np.asarray(inputs["beta2"], dtype=np.float32)
    w3 = np.asarray(inputs["w3"], dtype=np.float32)
    b3 = np.asarray(inputs["b3"], dtype=np.float32)
    table = np.asarray(inputs["importance_table"], dtype=np.float32)

    B, S_, H_ = hidden.shape
    assert (B, S_, H_) == (4, S, H), (B, S_, H_)

    x_full = hidden + pos                                      # [B, S, H]

    NC8 = H // P

    def perm_w(wT):
        # [H(in), H(out)] -> [head, p, c8, n]: row c8*128+p, col h*128+n
        return np.ascontiguousarray(
            wT.reshape(NC8, P, NH, HD).transpose(2, 1, 0, 3).astype(bf))

    # fold 1/sqrt(hd) into the q projection (weights and bias)
    wqR_ = perm_w(in_proj_w[0:H].T * INV_SQRT_HD)
    wkR_ = perm_w(in_proj_w[H:2 * H].T)
    wvR_ = np.ascontiguousarray(                               # [p, c8, d]
        in_proj_w[2 * H:3 * H].T.reshape(NC8, P, H)
        .transpose(1, 0, 2).astype(bf))
    bq = in_proj_b[0:H] * INV_SQRT_HD
    bk = in_proj_b[H:2 * H]
    bv = in_proj_b[2 * H:3 * H]
    # fold attention out-projection into the first meta layer:
    # w1 @ [x; att] + b1 == w1x @ x + (w1a @ out_w) @ ctx + (b1 + w1a @ out_b)
    w1x = w1[:, :H]
    w1a = w1[:, H:]
    w1eff = w1a @ out_w                                        # [MD, H]
    b1eff = b1 + w1a @ out_b
    w1cR_ = np.ascontiguousarray(                              # [p, j, md]
        np.concatenate([w1x, w1eff], axis=1).T
        .reshape(2 * NC8, P, MD).transpose(1, 0, 2).astype(bf))
    w2R_ = np.ascontiguousarray(                               # [p, ft, md2]
        w2.T.reshape(MD // P, P, MD2).transpose(1, 0, 2).astype(bf))

    def cmaj(v):   # [F] -> [128, F/128] partition-major
        return np.ascontiguousarray(v.reshape(-1, P).T)

    def bcast(v):  # [F] -> [128, F]
        return np.ascontiguousarray(np.broadcast_to(v[None, :], (P, v.shape[0])))

    def pack_consts(kb_arr, maskf_arr):
        cp = np.zeros((P, NCPK), dtype=np.float32)
        def put(name, arr):
            lo, hi = _CPK_SPANS[name]
            cp[:, lo:hi] = arr
        put("kbias", cmaj(kb_arr))
        put("maskf", maskf_arr)
        put("bq", cmaj(bq))
        put("bk", cmaj(bk))
        put("b1", cmaj(b1eff))
        put("g1", cmaj(g1))
        put("be1", cmaj(beta1))
        put("b3", np.full((P, 1), b3[0], dtype=np.float32))
        put("w3", bcast(w3[0]))
        put("b2", bcast(b2))
        put("g2", bcast(g2))
        put("be2", bcast(beta2))
        put("bv", bcast(bv))
        return cp

    shared = {
        "wqR": wqR_, "wkR": wkR_, "wvR": wvR_,
        "w1cR": w1cR_, "w2R": w2R_,
        "table": np.ascontiguousarray(table[:, None]),
    }

    in_maps = []
    for c in range(8):
        b = c // 2
        half = c % 2
        own = slice(half * SQ, (half + 1) * SQ)
        oth = slice((1 - half) * SQ, (2 - half) * SQ)
        xb = x_full[b].T                                       # [H, S] view
        # arrange so own half occupies columns [0, SQ); pre-permute to
        # [cb, p, c8, n] so each column-block DMA is contiguous
        xT_arr = np.concatenate([xb[:, own], xb[:, oth]], axis=1)
        xTr_arr = np.ascontiguousarray(
            xT_arr.reshape(NC8, P, S // 512, 512)
            .transpose(2, 1, 0, 3).astype(bf))
        kb = np.where(mask[b], 0.0, -1e9).astype(np.float32)
        kb_arr = np.concatenate([kb[own], kb[oth]])            # match column remap
        maskf_arr = np.ascontiguousarray(
            mask[b, own].astype(np.float32).reshape(-1, P).T)
        m = {
            "xTr": xTr_arr,
            "cpack": pack_consts(kb_arr, maskf_arr),
            "tokc": np.ascontiguousarray(token_ids[b, own].reshape(-1, P).T),
        }
        m.update(shared)
        in_maps.append(m)
    return in_maps


def _assemble(res):
    full = np.zeros((4, S), dtype=np.float32)
    for c in range(8):
        b = c // 2
        half = c % 2
        full[b, half * SQ:(half + 1) * SQ] = res.results[c]["out"]
    return full


def kernel(**inputs) -> np.ndarray:
    from concourse.bass_utils import run_bass_kernel_spmd
    in_maps = _prep_in_maps(inputs)
    nc = _get_program()
    res = run_bass_kernel_spmd(nc, in_maps, list(range(8)))
    return _assemble(res)


def run_traced(inputs, **kwargs):
    from concourse.bass_utils import run_bass_kernel_spmd
    in_maps = _prep_in_maps(inputs)
    nc = _get_program()
    return run_bass_kernel_spmd(nc, in_maps, list(range(8)), trace=True, **kwargs)


# revision 59
# speedup vs baseline: 1.1856x; 1.1856x over previous
"""Trainium2 Bass kernel for EnhancedMetaWeightNetwork.

Full (unsharded) inputs in, full output out. Internally: 8 NeuronCores,
core c handles batch b = c // 2 and query-row half c % 2 (1024 rows).
Attention K/V are computed per-core for the full sequence of the core's
batch (duplicated across the 2 cores sharing a batch; no collectives).

v3 layout strategy (bf16 storage, fp32 PSUM accumulation):
  - x = hidden + pos_embed precomputed on host; uploaded as bf16 x^T.
  - out_w folded into w1 on host (w1eff = w1[:, H:] @ out_w), so the
    attention out-projection disappears; h1 = w1x @ x + w1eff @ ctx.
  - everything SBUF-resident: no DRAM scratch.
  - fused per-head pipeline: K[0],Q[0] -> V (all) -> for each head:
    attention(h) interleaved with K/Q(h+1), so the scalar-engine exp
    stream hides under the PE matmul stream.
  - scoresT [key, query] per head in a 2-bank PSUM tile covering the
    full 1024 own queries; one exp activation per key-tile.
  - softmax denominator: pairwise ex adds on gpsimd + running
    accumulation on DVE, one ones-matmul broadcast, then
    reciprocal_approx_fast.
  - K/Q staging (PSUM -> bf16 SBUF + bias) on DVE to keep scalar free
    for exp; all constants packed into one DMA.
  - importance lookup via indirect DMA gather from the vocab table.
"""

import numpy as np

H = 1024
NH = 8
HD = 128           # head dim
S = 2048           # keys / full sequence
SQ = 1024          # own query rows per core
MD = 256           # meta dim
MD2 = 128
VOCAB = 32000
MIN_W, MAX_W = 0.1, 5.0
LN_EPS = 1e-5
P = 128
INV_SQRT_HD = 1.0 / np.sqrt(np.float32(HD))

# packed fp32 constant columns: [kbias 16 | maskf 8 | bq 8 | bk 8 | b1 2 |
#  g1 2 | be1 2 | b3 1 | w3 128 | b2 128 | g2 128 | be2 128 | bv 1024]
_CPK_SPANS = {}
_off = 0
for _name, _n in [("kbias", 16), ("maskf", 8), ("bq", 8), ("bk", 8),
                  ("b1", 2), ("g1", 2), ("be1", 2), ("b3", 1), ("w3", MD2),
                  ("b2", MD2), ("g2", MD2), ("be2", MD2), ("bv", H)]:
    _CPK_SPANS[_name] = (_off, _off + _n)
    _off += _n
NCPK = _off

_CACHE = {}


def _build():
    import concourse.bass as bass
    import concourse.mybir as mybir
    import concourse.tile as tile
    from concourse import bacc

    f32 = mybir.dt.float32
    bf16 = mybir.dt.bfloat16
    i32 = mybir.dt.int32
    OP = mybir.AluOpType
    ACT = mybir.ActivationFunctionType

    nc = bacc.Bacc("TRN2", target_bir_lowering=False, debug=False,
                   enable_asserts=False, num_devices=8)

    # ---------------- DRAM parameters ----------------
    dp = nc.declare_dram_parameter
    xT = dp("xT", [H, S], bf16, isOutput=False)           # (hidden+pos)[b].T, own half first
    wqT = dp("wqT", [H, H], bf16, isOutput=False)         # in_proj_w[0:H].T / sqrt(hd)
    wkT = dp("wkT", [H, H], bf16, isOutput=False)
    wvT = dp("wvT", [H, H], bf16, isOutput=False)
    w1cT = dp("w1cT", [2 * H, MD], bf16, isOutput=False)  # [w1x | w1a@out_w].T
    w2T = dp("w2T", [MD, MD2], bf16, isOutput=False)
    cpack = dp("cpack", [P, NCPK], f32, isOutput=False)
    tokc = dp("tokc", [P, SQ // P], i32, isOutput=False)
    table = dp("table", [VOCAB, 1], f32, isOutput=False)
    out = dp("out", [SQ], f32, isOutput=True)

    NKT = S // P          # 16 key tiles
    NC8 = H // P          # 8 feature chunks
    NTT = SQ // P         # 8 own token tiles
    NFT = MD // P         # 2 feature tiles of h1

    with tile.TileContext(nc) as tc:
        with tc.tile_pool(name="const", bufs=1) as cst, \
             tc.tile_pool(name="persist", bufs=1) as pa:

            # ---------------- constants (no DMA) ----------------
            ones_f = cst.tile([P, P], f32, tag="ones_f")
            nc.any.memset(ones_f[:], 1.0)
            ones_b = cst.tile([P, P], bf16, tag="ones_b")
            nc.vector.tensor_copy(ones_b[:], ones_f[:])
            eps_sb = cst.tile([P, 1], f32, tag="eps")
            nc.any.memset(eps_sb[:], LN_EPS)

            # ---- long-lived activations ----
            x_sb = pa.tile([P, NC8, S], bf16, tag="x")       # full x^T (own cols first)
            ctx_sb = pa.tile([P, NC8, SQ], bf16, tag="ctx")  # ctx^T (head-major)
            v_sb = pa.tile([P, NKT, H], bf16, tag="v")
            w1c_sb = pa.tile([P, 2 * NC8, MD], bf16, tag="w1c")
            h1p = pa.tile([P, NFT, SQ], bf16, tag="h1p")
            h1n = pa.tile([P, NFT, SQ], bf16, tag="h1n")
            res_sb = pa.tile([P, NTT], f32, tag="res")

            with tc.tile_pool(name="kqs", bufs=3) as kqs, \
                 tc.tile_pool(name="wst", bufs=2) as wst, \
                 tc.tile_pool(name="ps_kq", bufs=2, space="PSUM") as ps_kq:

                # ---- DMA issue order: head-0 weights, then x, then rest ----
                def load_w(src, h, tag):
                    t = wst.tile([P, NC8, P], bf16, tag=tag)
                    nc.sync.dma_start(t[:], src[:, h * P:(h + 1) * P]
                                      .rearrange("(c p) n -> p c n", p=P))
                    return t

                wk0 = load_w(wkT, 0, "wk")
                wq0 = load_w(wqT, 0, "wq")

                # load x by column blocks so K(0) can start after the first
                def load_x(cb):
                    nc.sync.dma_start(
                        x_sb[:, :, cb * 512:(cb + 1) * 512],
                        xT[:, cb * 512:(cb + 1) * 512]
                        .rearrange("(c p) n -> p c n", p=P))

                # small constants (biases etc.) before x so K staging never
                # waits; the big bv tail of cpack comes later
                cpk = cst.tile([P, NCPK], f32, tag="cpk")
                _bv_lo = _CPK_SPANS["bv"][0]
                nc.sync.dma_start(cpk[:, 0:_bv_lo], cpack[:, 0:_bv_lo])

                load_x(0)
                load_x(1)

                def cslice(name):
                    lo, hi = _CPK_SPANS[name]
                    return cpk[:, lo:hi]

                kbias_sb = cslice("kbias")
                maskf_sb = cslice("maskf")
                bq_sb = cslice("bq")
                bk_sb = cslice("bk")
                b1_c = cslice("b1")
                g1_c = cslice("g1")
                be1_c = cslice("be1")
                b3_sb = cslice("b3")
                w3_sb = cslice("w3")
                b2_sb = cslice("b2")
                g2_sb = cslice("g2")
                be2_sb = cslice("be2")
                bv_sb = cslice("bv")

                # ---- per-head K/Q projection + staging (DVE) ----
                def emit_k(h, wk_sb, k_h=None, sbs=None):
                    if k_h is None:
                        k_h = kqs.tile([P, S], bf16, tag="k_h")
                    for sb in (sbs if sbs is not None else range(S // 512)):
                        psk = ps_kq.tile([P, 512], mybir.dt.float32, tag="kq",
                                         name="psk")
                        for c8 in range(NC8):
                            nc.tensor.matmul(psk[:], lhsT=wk_sb[:, c8, :],
                                             rhs=x_sb[:, c8, sb * 512:(sb + 1) * 512],
                                             start=(c8 == 0), stop=(c8 == NC8 - 1))
                        nc.vector.tensor_tensor(
                            out=k_h[:, sb * 512:(sb + 1) * 512], in0=psk[:],
                            in1=bk_sb[:, h:h + 1].to_broadcast([P, 512]),
                            op=OP.add)
                    return k_h

                def emit_q(h, wq_sb):
                    q_h = kqs.tile([P, SQ], bf16, tag="q_h")
                    for qb in range(SQ // 512):
                        psq = ps_kq.tile([P, 512], mybir.dt.float32, tag="kq",
                                         name="psq")
                        for c8 in range(NC8):
                            nc.tensor.matmul(psq[:], lhsT=wq_sb[:, c8, :],
                                             rhs=x_sb[:, c8, qb * 512:(qb + 1) * 512],
                                             start=(c8 == 0), stop=(c8 == NC8 - 1))
                        nc.vector.tensor_tensor(
                            out=q_h[:, qb * 512:(qb + 1) * 512], in0=psq[:],
                            in1=bq_sb[:, h:h + 1].to_broadcast([P, 512]),
                            op=OP.add)
                    return q_h

                # ---- V weight / remaining loads interleaved with the
                # two up-front heads of K/Q (fills the x load window) ----
                with tc.tile_pool(name="wvp", bufs=1) as wvp, \
                     tc.tile_pool(name="ps_v", bufs=3, space="PSUM") as ps_v:
                    wv_sb = wvp.tile([P, NC8, H], bf16, tag="wv")
                    for db in range(H // 512):
                        nc.sync.dma_start(
                            wv_sb[:, :, db * 512:(db + 1) * 512],
                            wvT[:, db * 512:(db + 1) * 512]
                            .rearrange("(c p) n -> p c n", p=P))
                    load_x(2)
                    load_x(3)
                    wk1 = load_w(wkT, 1, "wk")
                    wq1 = load_w(wqT, 1, "wq")
                    nc.sync.dma_start(cpk[:, _bv_lo:], cpack[:, _bv_lo:])
                    tok_sb = cst.tile([P, NTT], i32, tag="tok")
                    nc.sync.dma_start(tok_sb[:], tokc[:])
                    w2_sb = cst.tile([P, NFT, MD2], bf16, tag="w2")
                    nc.sync.dma_start(w2_sb[:],
                                      w2T[:].rearrange("(c p) n -> p c n", p=P))
                    nc.sync.dma_start(w1c_sb[:],
                                      w1cT[:].rearrange("(c p) n -> p c n", p=P))

                    # importance gather (gpsimd queue; independent)
                    imp_all = cst.tile([P, NTT], f32, tag="imp_all")
                    for tt in range(NTT):
                        nc.gpsimd.indirect_dma_start(
                            out=imp_all[:, tt:tt + 1], out_offset=None,
                            in_=table[:],
                            in_offset=bass.IndirectOffsetOnAxis(
                                ap=tok_sb[:, tt:tt + 1], axis=0))

                    # head 0/1 K/Q; K0's other-half blocks after Q0 so the
                    # PE is not waiting on x blocks 2/3
                    k_next = emit_k(0, wk0, sbs=[0, 1])
                    q_next = emit_q(0, wq0)
                    emit_k(0, wk0, k_h=k_next, sbs=[2, 3])
                    k_next2 = emit_k(1, wk1)
                    q_next2 = emit_q(1, wq1)

                    for tt in range(NKT):
                        psv = ps_v.tile([P, 1024], mybir.dt.float32, tag="v2")
                        for c8 in range(NC8):
                            lhsT = x_sb[:, c8, tt * P:(tt + 1) * P]
                            for db in range(H // 512):
                                nc.tensor.matmul(psv[:, db * 512:(db + 1) * 512],
                                                 lhsT=lhsT,
                                                 rhs=wv_sb[:, c8, db * 512:(db + 1) * 512],
                                                 start=(c8 == 0), stop=(c8 == NC8 - 1))
                        nc.vector.tensor_tensor(out=v_sb[:, tt, :], in0=psv[:],
                                                in1=bv_sb[:], op=OP.add)

                # ---- fused attention + next-head K/Q pipeline ----
                with tc.tile_pool(name="exps", bufs=4) as exps, \
                     tc.tile_pool(name="trp", bufs=3) as trp, \
                     tc.tile_pool(name="trd", bufs=2) as trd, \
                     tc.tile_pool(name="rcps", bufs=2) as rcps, \
                     tc.tile_pool(name="ps_sc", bufs=2, space="PSUM") as ps_sc, \
                     tc.tile_pool(name="ps_cps", bufs=1, space="PSUM") as ps_cps:
                    for h in range(NH):
                        k_cur, q_cur = k_next, q_next
                        k_next, q_next = k_next2, q_next2
                        if h + 2 < NH:
                            wk_nx = load_w(wkT, h + 2, "wk")
                            wq_nx = load_w(wqT, h + 2, "wq")

                        cps = ps_cps.tile([P, 1024], mybir.dt.float32, tag="cps")
                        pair = None
                        acc = None
                        for kt in range(NKT):
                            sc = ps_sc.tile([P, 1024], mybir.dt.float32, tag="sc")
                            for qb in range(SQ // 512):
                                nc.tensor.matmul(
                                    sc[:, qb * 512:(qb + 1) * 512],
                                    lhsT=k_cur[:, kt * P:(kt + 1) * P],
                                    rhs=q_cur[:, qb * 512:(qb + 1) * 512],
                                    start=True, stop=True)
                            ex = exps.tile([P, 1024], bf16, tag="ex")
                            nc.scalar.activation(ex[:], sc[:], ACT.Exp,
                                                 bias=kbias_sb[:, kt:kt + 1],
                                                 scale=1.0)
                            for qb in range(SQ // 512):
                                nc.tensor.matmul(
                                    cps[:, qb * 512:(qb + 1) * 512],
                                    lhsT=v_sb[:, kt, h * P:(h + 1) * P],
                                    rhs=ex[:, qb * 512:(qb + 1) * 512],
                                    start=(kt == 0), stop=(kt == NKT - 1))
                            if kt % 2 == 0:
                                ex_even = ex
                            else:
                                # pairwise add + running accumulation on DVE
                                pair = trp.tile([P, 1024], bf16, tag="pair")
                                nc.vector.tensor_tensor(out=pair[:], in0=ex_even[:],
                                                        in1=ex[:], op=OP.add)
                                if kt == 1:
                                    acc = pair
                                else:
                                    nacc = trd.tile([P, 1024], bf16, tag="acc")
                                    nc.vector.tensor_tensor(out=nacc[:], in0=acc[:],
                                                            in1=pair[:], op=OP.add)
                                    acc = nacc

                        # denominator broadcast via ones-matmul, then approx
                        # reciprocal and context scale (emitted after K(h+1)
                        # so the PE never waits on the adder tree)
                        def finish_head(h, cps, acc):
                            rcb = rcps.tile([P, 1024], f32, tag="rcb")
                            for qb in range(SQ // 512):
                                dnf = ps_kq.tile([P, 512], mybir.dt.float32,
                                                 tag="kq", name="dnf")
                                nc.tensor.matmul(dnf[:], lhsT=ones_b[:],
                                                 rhs=acc[:, qb * 512:(qb + 1) * 512],
                                                 start=True, stop=True)
                                with nc.allow_low_precision(reason="softmax rcp"):
                                    nc.vector.reciprocal_approx_fast(
                                        out=rcb[:, qb * 512:(qb + 1) * 512],
                                        in_=dnf[:])
                            nc.vector.tensor_tensor(out=ctx_sb[:, h, :], in0=cps[:],
                                                    in1=rcb[:], op=OP.mult)

                        if h + 2 < NH:
                            k_next2 = emit_k(h + 2, wk_nx)
                            finish_head(h, cps, acc)
                            q_next2 = emit_q(h + 2, wq_nx)
                        else:
                            finish_head(h, cps, acc)

            # ---------- meta MLP ----------
            with tc.tile_pool(name="mw", bufs=1) as mw, \
                 tc.tile_pool(name="msml", bufs=2) as sml, \
                 tc.tile_pool(name="ps_m", bufs=6, space="PSUM") as ps2:
                # ---- h1 in feature-major: h1preT [256, SQ] (qb-outer so
                # LN1(qb0) overlaps the qb1 matmuls on the PE) ----
                for qb in range(SQ // 512):
                    qsl = slice(qb * 512, (qb + 1) * 512)
                    for ft in range(NFT):
                        psf_t = ps2.tile([P, 512], mybir.dt.float32,
                                         tag="mm512", name="psf")
                        for j in range(2 * NC8):
                            if j < NC8:
                                rhs = x_sb[:, j, qsl]
                            else:
                                rhs = ctx_sb[:, j - NC8, qsl]
                            nc.tensor.matmul(
                                psf_t[:],
                                lhsT=w1c_sb[:, j, ft * P:(ft + 1) * P],
                                rhs=rhs,
                                start=(j == 0), stop=(j == 2 * NC8 - 1))
                        nc.scalar.activation(
                            h1p[:, ft, qsl], psf_t[:],
                            ACT.Identity, bias=b1_c[:, ft:ft + 1], scale=1.0)
                    h1sqs = []
                    for ft in range(NFT):
                        h1sq = sml.tile([P, 512], bf16, tag=f"h1sq{ft}")
                        nc.vector.tensor_tensor(out=h1sq[:], in0=h1p[:, ft, qsl],
                                                in1=h1p[:, ft, qsl], op=OP.mult)
                        h1sqs.append(h1sq)
                    psA = ps2.tile([P, 512], mybir.dt.float32, tag="mm512",
                                   name="psA")
                    psB = ps2.tile([P, 512], mybir.dt.float32, tag="mm512",
                                   name="psB")
                    for ft in range(NFT):
                        nc.tensor.matmul(psA[:], lhsT=ones_b[:],
                                         rhs=h1p[:, ft, qsl],
                                         start=(ft == 0), stop=(ft == NFT - 1))
                    for ft in range(NFT):
                        nc.tensor.matmul(psB[:], lhsT=ones_b[:],
                                         rhs=h1sqs[ft][:],
                                         start=(ft == 0), stop=(ft == NFT - 1))
                    nmean = sml.tile([P, 512], f32, tag="nmean")
                    ex2m = sml.tile([P, 512], f32, tag="ex2m")
                    m2r = sml.tile([P, 512], f32, tag="m2r")
                    nc.vector.tensor_scalar_mul(nmean[:], psA[:], -1.0 / MD)
                    nc.vector.tensor_scalar_mul(ex2m[:], psB[:], 1.0 / MD)
                    nc.vector.tensor_tensor(out=m2r[:], in0=nmean[:],
                                            in1=nmean[:], op=OP.mult)
                    nc.vector.tensor_tensor(out=ex2m[:], in0=ex2m[:],
                                            in1=m2r[:], op=OP.subtract)
                    # rstd = exp(-0.5 * ln(var + eps)) on ACT (fast path)
                    nc.scalar.activation(ex2m[:], ex2m[:], ACT.Ln,
                                         bias=eps_sb[:, 0:1], scale=1.0)
                    rstd = sml.tile([P, 512], bf16, tag="rstd")
                    with nc.allow_low_precision(reason="bf16 layernorm scale"):
                        nc.scalar.activation(rstd[:], ex2m[:], ACT.Exp,
                                             bias=0.0, scale=-0.5)
                    for ft in range(NFT):
                        h1c = sml.tile([P, 512], bf16, tag=f"h1c{ft}")
                        nc.vector.tensor_tensor(out=h1c[:], in0=h1p[:, ft, qsl],
                                                in1=nmean[:], op=OP.add)
                        nc.vector.tensor_tensor(out=h1c[:], in0=h1c[:],
                                                in1=rstd[:], op=OP.mult)
                        nc.scalar.activation(h1n[:, ft, qsl], h1c[:],
                                             ACT.Relu, bias=be1_c[:, ft:ft + 1],
                                             scale=g1_c[:, ft:ft + 1])

                # ---- h2 + LN2/final, per-token-tile groups so the LN2
                # chains pipeline behind the h2 matmuls (short tail) ----
                F2 = float(MD2)
                NG = 2
                GT = NTT // NG      # 4 token tiles per group
                hb2s = []
                for g in range(NG):
                    hb2 = mw.tile([P, GT, MD2], f32, tag=f"hb2{g}")
                    for ti in range(GT):
                        tt = g * GT + ti
                        ph2_t = ps2.tile([P, 512], mybir.dt.float32, tag="mm512",
                                         name="ph2")
                        ph2 = ph2_t[:, :MD2]
                        for ft in range(NFT):
                            nc.tensor.matmul(ph2,
                                             lhsT=h1n[:, ft, tt * P:(tt + 1) * P],
                                             rhs=w2_sb[:, ft, :],
                                             start=(ft == 0), stop=(ft == NFT - 1))
                        nc.vector.scalar_tensor_tensor(out=hb2[:, ti, :], in0=ph2,
                                                       scalar=1.0, in1=b2_sb[:],
                                                       op0=OP.mult, op1=OP.add)
                    hb2s.append(hb2)
                # LN2/final chains for the two groups, op-interleaved so the
                # DVE pipeline hides each chain's dependency latency
                R = range(NG)
                def gtile(name, shape):
                    return [sml.tile(shape, f32, tag=f"{name}{g}",
                                     name=f"{name}{g}") for g in R]
                sums2 = gtile("sums2", [P, GT])
                msq = gtile("msq", [P, GT, MD2])
                ssq2 = gtile("ssq2", [P, GT])
                nm2 = gtile("nm2", [P, GT])
                ex22 = gtile("ex22", [P, GT])
                mm2v = gtile("mm2", [P, GT])
                var2 = gtile("var2", [P, GT])
                std2 = gtile("std2", [P, GT])
                rstd2 = gtile("rstd2", [P, GT])
                t1a = gtile("t1a", [P, GT, MD2])
                base8 = gtile("base8", [P, GT])
                imp1a = gtile("imp1a", [P, GT])
                for g in R:
                    nc.vector.tensor_scalar_add(imp1a[g][:],
                                                imp_all[:, g * GT:(g + 1) * GT],
                                                1.0)
                for g in R:
                    nc.vector.reduce_sum(sums2[g][:], hb2s[g][:],
                                         axis=mybir.AxisListType.X)
                for g in R:
                    nc.vector.tensor_tensor(out=msq[g][:], in0=hb2s[g][:],
                                            in1=hb2s[g][:], op=OP.mult)
                for g in R:
                    nc.vector.reduce_sum(ssq2[g][:], msq[g][:],
                                         axis=mybir.AxisListType.X)
                for g in R:
                    nc.vector.tensor_scalar_mul(nm2[g][:], sums2[g][:], -1.0 / F2)
                for g in R:
                    nc.vector.tensor_scalar_mul(ex22[g][:], ssq2[g][:], 1.0 / F2)
                for g in R:
                    nc.vector.tensor_tensor(out=mm2v[g][:], in0=nm2[g][:],
                                            in1=nm2[g][:], op=OP.mult)
                for g in R:
                    nc.vector.tensor_tensor(out=var2[g][:], in0=ex22[g][:],
                                            in1=mm2v[g][:], op=OP.subtract)
                for g in R:
                    nc.scalar.activation(std2[g][:], var2[g][:], ACT.Sqrt,
                                         bias=eps_sb[:, 0:1], scale=1.0)
                for g in R:
                    nc.vector.reciprocal(rstd2[g][:], std2[g][:])
                for g in R:
                    nc.vector.tensor_tensor(
                        out=t1a[g][:], in0=hb2s[g][:],
                        in1=nm2[g][:, :, None].to_broadcast([P, GT, MD2]),
                        op=OP.add)
                for g in R:
                    nc.vector.tensor_tensor(
                        out=t1a[g][:], in0=t1a[g][:],
                        in1=rstd2[g][:, :, None].to_broadcast([P, GT, MD2]),
                        op=OP.mult)
                for g in R:
                    nc.vector.tensor_tensor(
                        out=t1a[g][:], in0=t1a[g][:],
                        in1=g2_sb[:, None, :].to_broadcast([P, GT, MD2]),
                        op=OP.mult)
                for g in R:
                    nc.vector.tensor_tensor(
                        out=t1a[g][:], in0=t1a[g][:],
                        in1=be2_sb[:, None, :].to_broadcast([P, GT, MD2]),
                        op=OP.add)
                for g in R:
                    nc.vector.tensor_scalar_max(t1a[g][:], t1a[g][:], 0.0)
                for g in R:
                    nc.vector.tensor_tensor(
                        out=t1a[g][:], in0=t1a[g][:],
                        in1=w3_sb[:, None, :].to_broadcast([P, GT, MD2]),
                        op=OP.mult)
                for g in R:
                    nc.vector.reduce_sum(base8[g][:], t1a[g][:],
                                         axis=mybir.AxisListType.X)
                for g in R:
                    nc.vector.tensor_tensor(
                        out=base8[g][:], in0=base8[g][:],
                        in1=b3_sb[:, 0:1].to_broadcast([P, GT]), op=OP.add)
                for g in R:
                    nc.vector.tensor_tensor(out=base8[g][:], in0=base8[g][:],
                                            in1=imp1a[g][:], op=OP.mult)
                for g in R:
                    nc.vector.tensor_scalar(base8[g][:], base8[g][:], MAX_W,
                                            MIN_W, op0=OP.min, op1=OP.max)
                for g in R:
                    nc.vector.tensor_tensor(
                        out=res_sb[:, g * GT:(g + 1) * GT], in0=base8[g][:],
                        in1=maskf_sb[:, g * GT:(g + 1) * GT], op=OP.mult)
                nc.sync.dma_start(out[:].rearrange("(t p) -> p t", p=P),
                                  res_sb[:])

    nc.compile()
    return nc


def _get_program():
    if "nc" not in _CACHE:
        _CACHE["nc"] = _build()
    return _CACHE["nc"]


def _prep_in_maps(inputs):
    import ml_dtypes
    bf = ml_dtypes.bfloat16

    hidden = np.asarray(inputs["hidden_states"], dtype=np.float32)
    token_ids = np.asarray(inputs["token_ids"], dtype=np.int32)
    mask = np.asarray(inputs["attention_mask"]).astype(bool)
    pos = np.asarray(inputs["pos_embed"], dtype=np.float32)
    in_proj_w = np.asarray(inputs["in_proj_w"], dtype=np.float32)
    in_proj_b = np.asarray(inputs["in_proj_b"], dtype=np.float32)
    out_w = np.asarray(inputs["out_w"], dtype=np.float32)
    out_b = np.asarray(inputs["out_b"], dtype=np.float32)
    w1 = np.asarray(inputs["w1"], dtype=np.float32)
    b1 = np.asarray(inputs["b1"], dtype=np.float32)
    g1 = np.asarray(inputs["g1"], dtype=np.float32)
    beta1 = np.asarray(inputs["beta1"], dtype=np.float32)
    w2 = np.asarray(inputs["w2"], dtype=np.float32)
    b2 = np.asarray(inputs["b2"], dtype=np.float32)
    g2 = np.asarray(inputs["g2"], dtype=np.float32)
    beta2 = np.asarray(inputs["beta2"], dtype=np.float32)
    w3 = np.asarray(inputs["w3"], dtype=np.float32)
    b3 = np.asarray(inputs["b3"], dtype=np.float32)
    table = np.asarray(inputs["importance_table"], dtype=np.float32)

    B, S_, H_ = hidden.shape
    assert (B, S_, H_) == (4, S, H), (B, S_, H_)

    x_full = hidden + pos                                      # [B, S, H]
    # fold 1/sqrt(hd) into the q projection (weights and bias)
    wqT_ = np.ascontiguousarray(
        (in_proj_w[0:H].T * INV_SQRT_HD).astype(bf))           # [H, H]
    wkT_ = np.ascontiguousarray(in_proj_w[H:2 * H].T.astype(bf))
    wvT_ = np.ascontiguousarray(in_proj_w[2 * H:3 * H].T.astype(bf))
    bq = in_proj_b[0:H] * INV_SQRT_HD
    bk = in_proj_b[H:2 * H]
    bv = in_proj_b[2 * H:3 * H]
    # fold attention out-projection into the first meta layer:
    # w1 @ [x; att] + b1 == w1x @ x + (w1a @ out_w) @ ctx + (b1 + w1a @ out_b)
    w1x = w1[:, :H]
    w1a = w1[:, H:]
    w1eff = w1a @ out_w                                        # [MD, H]
    b1eff = b1 + w1a @ out_b
    w1cT = np.ascontiguousarray(
        np.concatenate([w1x, w1eff], axis=1).T.astype(bf))     # [2H, MD]
    w2T_ = np.ascontiguousarray(w2.T.astype(bf))               # [MD, MD2]

    def cmaj(v):   # [F] -> [128, F/128] partition-major
        return np.ascontiguousarray(v.reshape(-1, P).T)

    def bcast(v):  # [F] -> [128, F]
        return np.ascontiguousarray(np.broadcast_to(v[None, :], (P, v.shape[0])))

    def pack_consts(kb_arr, maskf_arr):
        cp = np.zeros((P, NCPK), dtype=np.float32)
        def put(name, arr):
            lo, hi = _CPK_SPANS[name]
            cp[:, lo:hi] = arr
        put("kbias", cmaj(kb_arr))
        put("maskf", maskf_arr)
        put("bq", cmaj(bq))
        put("bk", cmaj(bk))
        put("b1", cmaj(b1eff))
        put("g1", cmaj(g1))
        put("be1", cmaj(beta1))
        put("b3", np.full((P, 1), b3[0], dtype=np.float32))
        put("w3", bcast(w3[0]))
        put("b2", bcast(b2))
        put("g2", bcast(g2))
        put("be2", bcast(beta2))
        put("bv", bcast(bv))
        return cp

    shared = {
        "wqT": wqT_, "wkT": wkT_, "wvT": wvT_,
        "w1cT": w1cT, "w2T": w2T_,
        "table": np.ascontiguousarray(table[:, None]),
    }

    in_maps = []
    for c in range(8):
        b = c // 2
        half = c % 2
        own = slice(half * SQ, (half + 1) * SQ)
        oth = slice((1 - half) * SQ, (2 - half) * SQ)
        xb = x_full[b].T                                       # [H, S] view
        # arrange so own half occupies columns [0, SQ)
        xT_arr = np.ascontiguousarray(
            np.concatenate([xb[:, own], xb[:, oth]], axis=1).astype(bf))
        kb = np.where(mask[b], 0.0, -1e9).astype(np.float32)
        kb_arr = np.concatenate([kb[own], kb[oth]])            # match column remap
        maskf_arr = np.ascontiguousarray(
            mask[b, own].astype(np.float32).reshape(-1, P).T)
        m = {
            "xT": xT_arr,
            "cpack": pack_consts(kb_arr, maskf_arr),
            "tokc": np.ascontiguousarray(token_ids[b, own].reshape(-1, P).T),
        }
        m.update(shared)
        in_maps.append(m)
    return in_maps


def _assemble(res):
    full = np.zeros((4, S), dtype=np.float32)
    for c in range(8):
        b = c // 2
        half = c % 2
        full[b, half * SQ:(half + 1) * SQ] = res.results[c]["out"]
    return full


def kernel(**inputs) -> np.ndarray:
    from concourse.bass_utils import run_bass_kernel_spmd
    in_maps = _prep_in_maps(inputs)
    nc = _get_program()
    res = run_bass_kernel_spmd(nc, in_maps, list(range(8)))
    return _assemble(res)


def run_traced(inputs, **kwargs):
    from concourse.bass_utils import run_bass_kernel_spmd
    in_maps = _prep_in_maps(inputs)
    nc = _get_program()
    return run_bass_kernel_spmd(nc, in_maps, list(range(8)), trace=True, **kwargs)
